# revision 44
# baseline (speedup 1.0000x reference)
"""Trainium2 Bass kernel for nn_Block_55336358643145 (dense transformer block).

Head-sharded exact-causal attention over 8 cores, v3 (489us -> ~408us).

Row-shards the 4096 (batch*seq) rows contiguously: core c owns rows
512c..512(c+1) (cores 0-3 batch 0, 4-7 batch 1). Per core: LN1 ->
transpose -> fp8e4 DoubleRow K/Q projections (weights pre-scaled x16,
unscaled in the psum->sbuf activation; DR pairs two 128-deep
contraction tiles per instruction for 2x matmul throughput) emitted
chunk-major so chunk oc (128 dims = heads 2oc,2oc+1) is the AllToAll
payload for core oc. ONE merged k+q AllToAll re-shards to head-parallel
(core j holds q/k for heads 2j,2j+1 over all 4096 rows): the first data
collective of the kernel always ends at ~(launch skew + transfer) ~=
108us regardless of its size or enqueue time (measured: splitting k|q
or adding a leading barrier is strictly worse), so the payload that
attention gates on travels as one collective and everything else stays
out of its way. v (fp8-DR, with a per-head ones column) follows in a
second AllToAll hidden under the first attention round, which defers
all its attn@v matmuls. Attention is an exact causal triangle identical
on every core (shared triangular corner mask via DVE add, per-ktile
column-zero exp bias folded into the ACT exp, shrinking q-windows on
diagonal tiles), fp8 scores and fp8 attn@v; the av stream lags the
scores by 3 tiles; per round the ones-column denominator is copied off
PSUM, approx-reciprocal'd, gpsimd-broadcast and wide-multiplied (the
[1,N] full reciprocal of v1 cost 40us of 1-partition DVE latency).
Rounds run full k-tiles first and diagonals last so a round's first
exps never wait on the DVE tri-add while it drains the previous norm
chain. The attention phase is ACT-bound (exp ~78us) with <10us of
gaps. Outputs ship fp8 through the o AllToAll; out-proj is fp8-DR +
residual (scale on ACT, add on DVE), LN2 interleaves per row-tile, and
the 4x MLP stays bf16 at its N-cycle floor (fp8 there would blow the
2e-2 error gate: measured 1.50e-2 with this config vs 2.5e-2 with fp8
MLP). PE transposes copy out through DVE, not ACT (ACT is the busy
engine in both windows). Dummy matmul bursts warm the HAM clock gate
(1.2 -> 2.4GHz) at kernel start and again on kg's arrival so the first
scores run warm. w1 is prefetched during the o collective; weight loads
ride big strided DMAs spread across the sync/scalar trigger queues.
"""

import contextlib
import sys
import types

import numpy as np

import concourse.bass as bass
import concourse.tile as tile
from concourse import bacc, mybir
from concourse.bass_utils import run_bass_kernel_spmd

# run_bass_kernel_spmd imports antenv.axon_hooks when BASS_TRACE is set; on
# images without it, register a no-op stub so tracing degrades gracefully
# instead of crashing.
try:
    import antenv.axon_hooks  # noqa: F401
except ImportError:
    _stub = types.ModuleType("antenv.axon_hooks")
    _stub.get_axon_ntff_profile_hook = lambda: None
    _stub.set_axon_ntff_profile_hook = lambda h: None
    sys.modules["antenv.axon_hooks"] = _stub

F32 = mybir.dt.float32
BF16 = mybir.dt.bfloat16
FP8 = mybir.dt.float8e4
AF = mybir.ActivationFunctionType
ALU = mybir.AluOpType
DR = mybir.MatmulPerfMode.DoubleRow

B, S, D, H, HD, FF = 2, 2048, 1024, 16, 64, 4096
NCORE = 8
R = 512            # rows per core
DC = D // 128      # 8 d-chunks
GC = FF // 128     # 32 mlp hidden chunks
VW = H * (HD + 1)  # 1040: v with per-head ones column
CW = 4 * (HD + 1) * 2  # 520: v cols per partition in an A2A chunk
KQVW = 2 * R + CW  # 1544: merged k|q|v payload cols per partition
LN_EPS = 1e-5
JD = 25            # joined dim for the column-zero mask
NEG = -1.0e30
WS = 16.0          # fp8 weight pre-scale
GROUPS = [[0, 1, 2, 3, 4, 5, 6, 7]]

# precision/feature knobs (build-time)
CFG = dict(
    v_fp8=True,    # v projected in fp8-DR and shipped fp8
    o_fp8=True,    # attention output ships fp8; out-proj fp8-DR
    barrier=False,  # tiny leading A2A to absorb core launch skew (measured:
                    # it just wastes a ~45us serial slot on the CC engine)
)


def build_program(apply_bv, apply_ln1_gb, apply_ln2_gb):
    nc = bacc.Bacc("TRN2", target_bir_lowering=False, debug=False,
                   num_devices=NCORE)

    def inp(name, shape, dt=F32):
        return nc.dram_tensor(name, list(shape), dt, kind="ExternalInput").ap()

    o_dt = FP8 if CFG["o_fp8"] else BF16
    io = dict(
        hs=inp("hs", (R, D)),
        wk=inp("wk", (D, D), FP8), wq=inp("wq", (D, D), FP8),
        wv=inp("wv", (D, D), FP8 if CFG["v_fp8"] else BF16),
        wp=inp("wp", (D, D), FP8 if CFG["o_fp8"] else BF16),
        w1=inp("w1", (GC, 128, DC, 128), BF16), w2=inp("w2", (FF, D), BF16),
        aux=inp("aux", (128, 448)),
        rowaux=inp("rowaux", (1, 2 * D + 128), BF16),
        bvh2=inp("bvh2", (HD, 2)),
        ln1gb=inp("ln1gb", (2, D)), ln2gb=inp("ln2gb", (2, D)),
        out=nc.dram_tensor("out", [R, D], F32, kind="ExternalOutput").ap(),
    )

    with tile.TileContext(nc) as tc:
        _build(tc, io, apply_bv, apply_ln1_gb, apply_ln2_gb)
    nc.compile()
    return nc


def _build(tc, io, apply_bv, apply_ln1_gb, apply_ln2_gb):
    nc = tc.nc
    hs, out = io["hs"], io["out"]
    v_dt = FP8 if CFG["v_fp8"] else BF16
    o_dt = FP8 if CFG["o_fp8"] else BF16

    with contextlib.ExitStack() as ctx:
        persist = ctx.enter_context(tc.tile_pool(name="persist", bufs=1,
                                                 side="left"))
        dram = ctx.enter_context(tc.tile_pool(name="dram", bufs=1,
                                              space="DRAM"))

        # ---- batched constants ---------------------------------------------
        # aux: [128, 448] f32 = bq8[0:8] bkl[8:16] b1l[16:48] colz[48:64]
        #                       tri[64:192] tri[192:320] ident[320:448]
        aux_sb = persist.tile([128, 448], F32)
        nc.sync.dma_start(aux_sb[:], io["aux"][:])
        bq8_sb = aux_sb[:, 0:8]
        bkl_sb = aux_sb[:, 8:16]
        b1l_sb = aux_sb[:, 16:48]
        colz_sb = aux_sb[:, 48:64]
        tri_sb = aux_sb[:, 64:320].rearrange("p (j q) -> p j q", j=2)
        ident_sb = aux_sb[:, 320:448]
        # rowaux: [1, 2D+128] bf16 = bpr[0:D] b2r[D:2D] onesr[2D:2D+128]
        rowaux_sb = persist.tile([1, 2 * D + 128], BF16)
        nc.sync.dma_start(rowaux_sb[:], io["rowaux"][:])
        bpr_sb = rowaux_sb[:, 0:D]
        b2r_sb = rowaux_sb[:, D:2 * D]
        ones_r = rowaux_sb[:, 2 * D:2 * D + 128]
        eps_sb = persist.tile([128, 1], F32)
        nc.vector.memset(eps_sb[:], LN_EPS)
        if apply_bv:
            bvh2_sb = persist.tile([HD, 2], F32)
            nc.sync.dma_start(bvh2_sb[:], io["bvh2"][:])

        def ln_gb_tiles(gb_inp, nm):
            g_sb = persist.tile([128, D], F32, name=f"g_{nm}")
            b_sb = persist.tile([128, D], F32, name=f"b_{nm}")
            g_row = persist.tile([1, D], F32, name=f"gr_{nm}")
            b_row = persist.tile([1, D], F32, name=f"br_{nm}")
            nc.sync.dma_start(g_row[:], gb_inp[0:1, :])
            nc.sync.dma_start(b_row[:], gb_inp[1:2, :])
            nc.gpsimd.partition_broadcast(g_sb[:], g_row[:])
            nc.gpsimd.partition_broadcast(b_sb[:], b_row[:])
            return g_sb, b_sb

        ln1_g = ln1_b = ln2_g = ln2_b = None
        if apply_ln1_gb:
            ln1_g, ln1_b = ln_gb_tiles(io["ln1gb"], "ln1")
        if apply_ln2_gb:
            ln2_g, ln2_b = ln_gb_tiles(io["ln2gb"], "ln2")

        # ---- DRAM collective buffers ---------------------------------------
        v_loc = dram.tile([NCORE, 128, CW], FP8)
        v_g = dram.tile([NCORE, 128, CW], FP8)
        kq_loc = dram.tile([NCORE, 128, 2 * R], FP8)
        kq_g = dram.tile([NCORE, 128, 2 * R], FP8)
        o_loc = dram.tile([NCORE, 128, R], o_dt)
        o_g = dram.tile([NCORE, 128, R], o_dt)
        if CFG["barrier"]:
            bar_loc = dram.tile([NCORE, 1, 16], FP8)
            bar_g = dram.tile([NCORE, 1, 16], FP8)
            bar_sb = persist.tile([1, NCORE * 16], FP8)
            nc.vector.memset(bar_sb[:], 1.0)
            nc.sync.dma_start(
                bar_loc[:].rearrange("r p q -> p (r q)"), bar_sb[:])
            nc.gpsimd.collective_compute(
                "AllToAll", ALU.bypass, replica_groups=GROUPS,
                ins=[bar_loc.opt()], outs=[bar_g.opt()])

        # ---- big input DMAs: hs on sync, weights spread over engines -------
        hs_pool = ctx.enter_context(contextlib.ExitStack())   # hs_sb: P0..P5
        hsp = hs_pool.enter_context(tc.tile_pool(name="hs_p", bufs=1,
                                                 side="right"))
        hs_sb = hsp.tile([128, 4, D], F32)
        for rt in range(4):
            nc.sync.dma_start(hs_sb[:, rt, :],
                              hs[128 * rt:128 * (rt + 1), :])

        es_w = ctx.enter_context(contextlib.ExitStack())
        wq_pool = es_w.enter_context(tc.tile_pool(name="wqkv", bufs=1,
                                                  side="left"))
        wk_t = wq_pool.tile([128, DC, D], FP8)
        nc.scalar.dma_start(wk_t[:], io["wk"].rearrange("(c p) j -> p c j",
                                                        p=128))
        wq_t = wq_pool.tile([128, DC, D], FP8)
        nc.scalar.dma_start(wq_t[:], io["wq"].rearrange("(c p) j -> p c j",
                                                        p=128))
        wv_t = wq_pool.tile([128, DC, D], v_dt)
        nc.sync.dma_start(wv_t[:], io["wv"].rearrange("(c p) j -> p c j",
                                                      p=128))
        # wp loaded up front in the same pool: its own SBUF (no WAR on the
        # qkv weights) and a sync-queue trigger (a scalar-queue trigger here
        # deadlocks the ACT engine behind the projection ACTIVATEs)
        wp_dt = FP8 if CFG["o_fp8"] else BF16
        wp_t = wq_pool.tile([128, DC, D], wp_dt)
        nc.sync.dma_start(wp_t[:], io["wp"].rearrange("(c p) j -> p c j",
                                                      p=128))

        def layernorm(dst, src, pool, g_sb, b_sb):
            stats = pool.tile([128, 2, 6], F32, tag="ln_stats")
            sg = src.rearrange("p (g d) -> p g d", g=2)
            for g in range(2):
                nc.vector.bn_stats(out=stats[:, g, :], in_=sg[:, g, :])
            mv = pool.tile([128, 2], F32, tag="ln_mv")
            nc.vector.bn_aggr(out=mv[:], in_=stats[:])
            rstd = pool.tile([128, 1], F32, tag="ln_rstd")
            nc.scalar.activation(out=rstd[:], in_=mv[:, 1:2], func=AF.Sqrt,
                                 bias=eps_sb[:], scale=1.0)
            nc.vector.reciprocal(out=rstd[:], in_=rstd[:])
            nc.vector.tensor_scalar(out=dst, in0=src, scalar1=mv[:, 0:1],
                                    scalar2=rstd[:], op0=ALU.subtract,
                                    op1=ALU.mult)
            if g_sb is not None:
                nc.vector.tensor_mul(dst, dst, g_sb[:])
                nc.vector.tensor_add(dst, dst, b_sb[:])

        def transpose_into(dstT, src_tile, rt, tp_pool):
            # psum->sbuf copies on DVE: the ACT queue is the busier engine
            # in both windows where this runs (k/q ACTIVATEs, gelu)
            for c in range(DC):
                tp = tp_pool.tile([128, 128], F32, tag="tp")
                nc.tensor.transpose(tp[:], src_tile[:, 128 * c:128 * (c + 1)],
                                    ident_sb)
                nc.vector.tensor_copy(dstT[:, c, 128 * rt:128 * (rt + 1)],
                                      tp[:])

        es_x = ctx.enter_context(contextlib.ExitStack())      # xT lifetime
        xT_pool = es_x.enter_context(
            tc.tile_pool(name="xT_p", bufs=1, side="left"))
        xT = xT_pool.tile([128, DC, R], FP8)

        # ================= P0: LN1 + transpose ===============================
        with tc.tile_pool(name="p0", bufs=2, side="left") as p0, \
             tc.tile_pool(name="p0ps", bufs=4, space="PSUM") as p0ps:
            # HAM warm-up: ~4us of back-to-back PE work as soon as the
            # identity lands, so P0/P1 run at full clock instead of the
            # cold 1.2 GHz default (measured: first 39us ran cold)
            wps = p0ps.tile([128, 128], F32, tag="tp", name="warm_ps")
            for _ in range(36):
                nc.tensor.transpose(wps[:], ident_sb, ident_sb)
            for rt in range(4):
                xln = p0.tile([128, D], F32, tag="xln")
                layernorm(xln[:], hs_sb[:, rt, :], p0, ln1_g, ln1_b)
                transpose_into(xT, xln, rt, p0ps)

        # ================= P1: k, q projections -> A2A(kq); then v ==========
        # kq is the FIRST collective: attention gates on it, and the first
        # collective of the kernel eats the CC cold-start + launch skew
        # (~30-45us) no matter its size. v's A2A runs second, hidden under
        # the first attention round (which defers all its attn@v matmuls).
        with tc.tile_pool(name="t_kq", bufs=3, side="left") as kq_tpl, \
             tc.tile_pool(name="ps_kq", bufs=2, space="PSUM") as kq_pps:
            for nm, wt, bias, scale, off in (
                    ("wk", wk_t, bkl_sb, 1.0 / WS, 0),
                    ("wq", wq_t, bq8_sb, 0.125 / WS, R)):
                for oc in range(DC):
                    ps = kq_pps.tile([128, R], F32, tag="ps",
                                     name=f"ps_{nm}_{oc}")
                    for g in range(4):
                        nc.tensor.matmul(
                            ps[:], wt[:, 2 * g:2 * g + 2,
                                      128 * oc:128 * (oc + 1)],
                            xT[:, 2 * g:2 * g + 2, :],
                            start=(g == 0), stop=(g == 3), perf_mode=DR)
                    tmp = kq_tpl.tile([128, R], FP8, tag="tmp",
                                      name=f"t_{nm}_{oc}")
                    nc.scalar.activation(tmp[:], ps[:], func=AF.Identity,
                                         bias=bias[:, oc:oc + 1], scale=scale)
                    nc.sync.dma_start(kq_loc[oc][:, off:off + R], tmp[:])
            # ONE merged k+q collective: splitting measures worse — the
            # first data collective ends at ~(launch skew + transfer) no
            # matter its size, and extra collectives serialize after it
            nc.gpsimd.collective_compute(
                "AllToAll", ALU.bypass, replica_groups=GROUPS,
                ins=[kq_loc.opt()], outs=[kq_g.opt()])

            # v: row-tile-major with interleaved per-head ones columns
            with tc.tile_pool(name="vaug_p", bufs=1, side="right") as vaug_p:
                vaug = vaug_p.tile([128, 4, VW], v_dt)
                nc.vector.memset(
                    vaug[:].rearrange("p t (h e) -> p t h e", e=HD + 1)
                    [:, :, :, HD:HD + 1], 1.0)
                for pt in range(4):
                    for cg in range(2):
                        ps = kq_pps.tile([128, 512], F32, tag="ps",
                                         name=f"ps_wv_{pt}_{cg}")
                        if CFG["v_fp8"]:
                            for g in range(4):
                                nc.tensor.matmul(
                                    ps[:],
                                    xT[:, 2 * g:2 * g + 2,
                                       128 * pt:128 * (pt + 1)],
                                    wv_t[:, 2 * g:2 * g + 2,
                                         512 * cg:512 * (cg + 1)],
                                    start=(g == 0), stop=(g == 3),
                                    perf_mode=DR)
                        else:
                            for c in range(DC):
                                nc.tensor.matmul(
                                    ps[:], xT[:, c, 128 * pt:128 * (pt + 1)],
                                    wv_t[:, c, 512 * cg:512 * (cg + 1)],
                                    start=(c == 0), stop=(c == DC - 1))
                        dst = vaug[:, pt, 520 * cg:520 * (cg + 1)].rearrange(
                            "p (h e) -> p h e", e=HD + 1)[:, :, 0:HD]
                        nc.scalar.activation(
                            dst, ps[:].rearrange("p (h e) -> p h e", e=HD),
                            func=AF.Identity,
                            scale=(1.0 / WS) if CFG["v_fp8"] else 1.0)
                for oc in range(DC):
                    nc.sync.dma_start(
                        v_loc[oc][:].rearrange(
                            "p (t e) -> p t e", e=2 * (HD + 1)),
                        vaug[:, :, 2 * (HD + 1) * oc:2 * (HD + 1) * (oc + 1)])

        nc.gpsimd.collective_compute(
            "AllToAll", ALU.bypass, replica_groups=GROUPS,
            ins=[v_loc.opt()], outs=[v_g.opt()])
        es_x.close()   # xT no longer needed

        # ================= P4: head-sharded causal attention =================
        es_attn = ctx.enter_context(contextlib.ExitStack())
        ao_pool = es_attn.enter_context(tc.tile_pool(name="ao_p", bufs=1,
                                                     side="left"))
        attn_oT = ao_pool.tile([128, DC, R], o_dt)
        with tc.tile_pool(name="kg_p", bufs=1, side="left") as kgp, \
             tc.tile_pool(name="vg_p", bufs=1, side="left") as vgp, \
             tc.tile_pool(name="qg_p", bufs=1, side="left") as qgp, \
             tc.tile_pool(name="ex_p", bufs=24, side="left") as exp_pool, \
             tc.tile_pool(name="nrm_p", bufs=4, side="left") as nrm, \
             tc.tile_pool(name="sc_ps", bufs=2, space="PSUM") as scps, \
             tc.tile_pool(name="oT_ps", bufs=4, space="PSUM") as otps:
            kg = kgp.tile([128, NCORE, R], FP8)
            qg = qgp.tile([128, NCORE, R], FP8)
            vg = vgp.tile([128, NCORE, 4, 2 * (HD + 1)], v_dt)
            nc.sync.dma_start(kg[:], kq_g[:, :, 0:R].rearrange(
                "r p q -> p r q"))
            nc.sync.dma_start(qg[:], kq_g[:, :, R:2 * R].rearrange(
                "r p q -> p r q"))
            nc.sync.dma_start(vg[:], v_g[:].rearrange(
                "r p (t e) -> p r t e", e=2 * (HD + 1)))
            # HAM re-warm: the PE sat idle through the kq collective and
            # would run the first ~14us of scores at 1.2 GHz; burn ~4us of
            # dummy transposes on kg (so they start the moment it lands,
            # overlapping the qg/vg loads) to unthrottle first
            w2ps = scps.tile([128, 128], F32, tag="sc", name="warm2_ps")
            for _ in range(36):
                nc.tensor.matmul(w2ps[:], kg[:, 0, 0:128], kg[:, 0, 0:128],
                                 start=True, stop=True)

            hps = (slice(0, 64), slice(64, 128))
            pending = []   # deferred av emissions; av lags sc by AV_LAG tiles
            AV_LAG = 3
            # first round: defer ALL avs — its score stream needs only k+q
            # (first A2A) and runs while the v A2A is still in flight
            lag = [99]

            def emit_av(f):
                is_last, norm_f = f()
                if is_last:
                    norm_f()

            def flush_av(upto):
                while len(pending) > upto:
                    emit_av(pending.pop(0))

            for B2 in range(2):
                for a in (3, 2, 1, 0):
                    rq = 4 * B2 + a
                    oTs = [otps.tile([HD + 1, R], F32, tag="oT",
                                     name=f"oT_{B2}_{a}_{j}")
                           for j in range(2)]
                    nkt = 4 * a + 4
                    # full tiles first, diagonals last: a diag tile's exp
                    # waits on the DVE tri-add, and at a round boundary DVE
                    # is still draining the previous round's norm chain
                    order = list(range(4 * a)) + list(range(4 * a, nkt))
                    exs = {}

                    def norm(oTs=oTs, B2=B2, a=a, rq=rq):
                        """normalize by the ones-column denominator, ship."""
                        o_sb = nrm.tile([128, R], o_dt, tag="osb",
                                        name=f"osb_{B2}_{a}")
                        for j in range(2):
                            # den to SBUF, fast-approx 1/den (SBUF-only op),
                            # broadcast (gpsimd can't read PSUM), wide mul
                            dc_ = nrm.tile([1, R], F32, tag="dc",
                                           name=f"dc_{B2}_{a}_{j}")
                            nc.vector.tensor_copy(dc_[:],
                                                  oTs[j][HD:HD + 1, :])
                            rc = nrm.tile([1, R], F32, tag="rc",
                                          name=f"rc_{B2}_{a}_{j}")
                            nc.vector.reciprocal_approx_fast(rc[:], dc_[:])
                            db = nrm.tile([HD, R], F32, tag="db",
                                          name=f"db_{B2}_{a}_{j}")
                            nc.gpsimd.partition_broadcast(db[:], rc[:])
                            nc.vector.tensor_mul(o_sb[hps[j], :],
                                                 oTs[j][0:HD, :], db[:])
                            if apply_bv:
                                nc.vector.tensor_scalar_add(
                                    o_sb[hps[j], :], o_sb[hps[j], :],
                                    bvh2_sb[:, j:j + 1])
                        nc.sync.dma_start(o_loc[rq], o_sb[:])

                    def av(i2, oTs=oTs, B2=B2, a=a, order=order, nkt=nkt,
                           exs=exs, norm=norm):
                        kt2 = order[i2]
                        rk2, t2 = 4 * B2 + kt2 // 4, kt2 % 4
                        d2 = kt2 - 4 * a
                        c2 = 128 * d2 if d2 >= 0 else 0
                        ex2 = exs.pop(i2)
                        for j in range(2):
                            nc.tensor.matmul(
                                oTs[j][:, c2:R],
                                vg[:, rk2, t2,
                                   (HD + 1) * j:(HD + 1) * (j + 1)],
                                ex2[:, j, c2:R],
                                start=(i2 == 0), stop=(i2 == nkt - 1))
                        return i2 == nkt - 1, norm

                    for i, kt in enumerate(order):
                        rk, t = 4 * B2 + kt // 4, kt % 4
                        d = kt - 4 * a
                        col0 = 128 * d if d >= 0 else 0
                        sc = scps.tile([128, 2, R], F32, tag="sc",
                                       name=f"sc_{B2}_{a}_{kt}")
                        for j in range(2):
                            nc.tensor.matmul(
                                sc[:, j, col0:R],
                                kg[hps[j], rk, 128 * t:128 * (t + 1)],
                                qg[hps[j], rq, col0:R],
                                start=True, stop=True)
                        if d >= 0:
                            nc.vector.tensor_add(sc[:, :, col0:col0 + 128],
                                                 sc[:, :, col0:col0 + 128],
                                                 tri_sb)
                        ex = exp_pool.tile([128, 2, R], v_dt, tag="ex",
                                           name=f"ex_{B2}_{a}_{kt}")
                        nc.scalar.activation(ex[:, :, col0:R],
                                             sc[:, :, col0:R], func=AF.Exp,
                                             bias=colz_sb[:, kt:kt + 1],
                                             scale=1.0)
                        exs[i] = ex
                        pending.append(lambda i=i, av=av: av(i))
                        flush_av(lag[0])
                    lag[0] = AV_LAG
            flush_av(0)

        nc.gpsimd.collective_compute(
            "AllToAll", ALU.bypass, replica_groups=GROUPS,
            ins=[o_loc.opt()], outs=[o_g.opt()])

        # w1 prefetch: manual double-buffer handles so the first 4 chunk
        # loads fire during the o collective / attention tail
        es_mw = ctx.enter_context(contextlib.ExitStack())
        w1pl = es_mw.enter_context(tc.tile_pool(name="w_w1", bufs=1,
                                                side="right"))
        W1BUF = 4
        w1_tiles = [w1pl.tile([128, DC, 128], BF16, name=f"w1t_{i}")
                    for i in range(W1BUF)]
        for i in range(W1BUF):
            nc.sync.dma_start(w1_tiles[i][:], io["w1"][i])

        nc.sync.dma_start(attn_oT[:],
                          o_g[:].rearrange("r p q -> p r q"))

        # ================= P5: out-proj + residual + LN2 =====================
        es_h = ctx.enter_context(contextlib.ExitStack())      # h_sb: P5..P8
        h_pool = es_h.enter_context(tc.tile_pool(name="h_p", bufs=1,
                                                 side="right"))
        h_sb = h_pool.tile([128, 4, D], F32)
        es_mlp = ctx.enter_context(contextlib.ExitStack())    # h2T, gT
        mlp_pool = es_mlp.enter_context(tc.tile_pool(name="mlp_p", bufs=1,
                                                     side="right"))
        h2T = mlp_pool.tile([128, DC, R], BF16)
        gT = mlp_pool.tile([128, GC, R], BF16)
        with tc.tile_pool(name="ps_wp", bufs=2, space="PSUM") as pps, \
             tc.tile_pool(name="p6", bufs=2, side="left") as p6, \
             tc.tile_pool(name="p6ps", bufs=4, space="PSUM") as p6ps:
            for rt in range(4):
                for cg in range(2):
                    ps = pps.tile([128, 512], F32, tag="ps",
                                  name=f"ps_wp_{rt}_{cg}")
                    if CFG["o_fp8"]:
                        for g in range(4):
                            nc.tensor.matmul(
                                ps[:],
                                attn_oT[:, 2 * g:2 * g + 2,
                                        128 * rt:128 * (rt + 1)],
                                wp_t[:, 2 * g:2 * g + 2,
                                     512 * cg:512 * (cg + 1)],
                                start=(g == 0), stop=False, perf_mode=DR)
                    else:
                        for c in range(DC):
                            nc.tensor.matmul(
                                ps[:], attn_oT[:, c, 128 * rt:128 * (rt + 1)],
                                wp_t[:, c, 512 * cg:512 * (cg + 1)],
                                start=(c == 0), stop=False)
                    nc.tensor.matmul(ps[:], ones_r,
                                     bpr_sb[:, 512 * cg:512 * (cg + 1)],
                                     start=False, stop=True)
                    if CFG["o_fp8"]:
                        # scale on ACT (idle here), add on DVE — fusing both
                        # into one DVE op measured slower (DVE is P5's busy
                        # engine: LN2 + adds + transpose copies)
                        tmp = p6.tile([128, 512], F32, tag="wtmp",
                                      name=f"wtmp_{rt}_{cg}")
                        nc.scalar.activation(tmp[:], ps[:], func=AF.Identity,
                                             scale=1.0 / WS)
                        nc.vector.tensor_add(
                            h_sb[:, rt, 512 * cg:512 * (cg + 1)], tmp[:],
                            hs_sb[:, rt, 512 * cg:512 * (cg + 1)])
                    else:
                        nc.vector.tensor_add(
                            h_sb[:, rt, 512 * cg:512 * (cg + 1)], ps[:],
                            hs_sb[:, rt, 512 * cg:512 * (cg + 1)])
                # LN2 + transpose of this row-tile overlaps the next one's
                # projection matmuls
                h2 = p6.tile([128, D], F32, tag="h2")
                layernorm(h2[:], h_sb[:, rt, :], p6, ln2_g, ln2_b)
                transpose_into(h2T, h2, rt, p6ps)
        es_attn.close()  # attn_oT done
        es_w.close()     # qkv + wp weights done

        # ================= P7: MLP up + gelu (bf16) ==========================
        with tc.tile_pool(name="ps_w1", bufs=2, space="PSUM") as pps:
            for gc in range(GC):
                wt = w1_tiles[gc % W1BUF]
                ps = pps.tile([128, R], F32, tag="ps", name=f"ps_w1_{gc}")
                for c in range(DC):
                    nc.tensor.matmul(ps[:], wt[:, c, :], h2T[:, c, :],
                                     start=(c == 0), stop=(c == DC - 1))
                if gc + W1BUF < GC:
                    nc.sync.dma_start(wt[:], io["w1"][gc + W1BUF])
                nc.scalar.activation(gT[:, gc, :], ps[:], func=AF.Gelu,
                                     bias=b1l_sb[:, gc:gc + 1], scale=1.0)

        # ================= P8: MLP down + bias + residual (bf16) =============
        with tc.tile_pool(name="w_w2", bufs=3, side="left") as wpl, \
             tc.tile_pool(name="o_sb", bufs=2, side="left") as osb, \
             tc.tile_pool(name="o_ps", bufs=1, space="PSUM") as pps:
            psts = [pps.tile([128, 512], F32, tag=f"o{i}", name=f"o_ps_{i}")
                    for i in range(8)]
            for gc in range(GC):
                wt = wpl.tile([128, D], BF16, tag="w2")
                nc.sync.dma_start(wt[:], io["w2"][128 * gc:128 * (gc + 1), :])
                for qt in range(4):
                    for cg in range(2):
                        nc.tensor.matmul(
                            psts[2 * qt + cg][:],
                            gT[:, gc, 128 * qt:128 * (qt + 1)],
                            wt[:, 512 * cg:512 * (cg + 1)],
                            start=(gc == 0), stop=False)
            for qt in range(4):
                ot = osb.tile([128, D], F32, tag="ot", name=f"ot_{qt}")
                for cg in range(2):
                    nc.tensor.matmul(psts[2 * qt + cg][:], ones_r,
                                     b2r_sb[:, 512 * cg:512 * (cg + 1)],
                                     start=False, stop=True)
                    nc.vector.tensor_add(ot[:, 512 * cg:512 * (cg + 1)],
                                         psts[2 * qt + cg][:],
                                         h_sb[:, qt, 512 * cg:512 * (cg + 1)])
                nc.sync.dma_start(out[128 * qt:128 * (qt + 1), :], ot[:])


# ---------------------------------------------------------------------------
# Host side
# ---------------------------------------------------------------------------

_CACHE = {}
LAST_RESULT = None  # BassKernelResults of the most recent run (for test.py)


def _get_program(key):
    if key not in _CACHE:
        _CACHE[key] = build_program(*key)
    return _CACHE[key]


def kernel(hidden_states, Wq, bq, Wk, bk, Wv, bv, Wp, bp,
           ln1_g, ln1_b, ln2_g, ln2_b, W1, b1, W2, b2):
    import ml_dtypes

    f32 = lambda a: np.ascontiguousarray(np.asarray(a, dtype=np.float32))
    hidden_states = f32(hidden_states)
    Wq, bq, Wk, bk, Wv, bv, Wp, bp = map(f32, (Wq, bq, Wk, bk, Wv, bv, Wp, bp))
    ln1_g, ln1_b, ln2_g, ln2_b = map(f32, (ln1_g, ln1_b, ln2_g, ln2_b))
    W1, b1, W2, b2 = map(f32, (W1, b1, W2, b2))

    apply_bv = bool(np.any(bv != 0.0))
    apply_ln1 = bool(np.any(ln1_g != 1.0) or np.any(ln1_b != 0.0))
    apply_ln2 = bool(np.any(ln2_g != 1.0) or np.any(ln2_b != 0.0))
    nc = _get_program((apply_bv, apply_ln1, apply_ln2))

    chunk_major = lambda v: np.ascontiguousarray(v.reshape(-1, 128).T)

    # triangular mask: within a diagonal window, q-col j attends kpos p iff
    # j >= p
    p = np.arange(128)[:, None]
    j = np.arange(128)[None, :]
    tri = np.where(j >= p, np.float32(0.0), np.float32(NEG))

    # per-ktile column-zero exp bias: kpos = 128*kt + p
    kt = np.arange(16)[None, :]
    kpos = 128 * kt + p
    colz = np.where((kpos % JD) == (JD - 1), np.float32(NEG), np.float32(0.0))

    bf = lambda a: np.ascontiguousarray(a.astype(ml_dtypes.bfloat16))
    f8 = lambda a, s=1.0: np.ascontiguousarray(
        (np.asarray(a, np.float32) * s).astype(ml_dtypes.float8_e4m3))
    w1x = np.ascontiguousarray(
        W1.reshape(DC, 128, GC, 128).transpose(2, 1, 0, 3))

    # aux pack: [128, 448] f32
    aux = np.zeros((128, 448), np.float32)
    aux[:, 0:8] = chunk_major(bq * 0.125)
    aux[:, 8:16] = chunk_major(bk)
    aux[:, 16:48] = chunk_major(b1)
    aux[:, 48:64] = colz
    aux[:, 64:192] = tri
    aux[:, 192:320] = tri
    aux[:, 320:448] = np.eye(128, dtype=np.float32)
    # rowaux: [1, 2D+128] bf16
    rowaux = np.zeros((1, 2 * D + 128), np.float32)
    rowaux[0, 0:D] = bp * (WS if CFG["o_fp8"] else 1.0)
    rowaux[0, D:2 * D] = b2
    rowaux[0, 2 * D:] = 1.0

    shared = dict(
        wk=f8(Wk, WS), wq=f8(Wq, WS),
        wv=f8(Wv, WS) if CFG["v_fp8"] else bf(Wv),
        wp=f8(Wp, WS) if CFG["o_fp8"] else bf(Wp),
        w1=bf(w1x), w2=bf(W2),
        aux=np.ascontiguousarray(aux), rowaux=bf(rowaux),
        ln1gb=np.stack([ln1_g, ln1_b]), ln2gb=np.stack([ln2_g, ln2_b]))

    hs_flat = hidden_states.reshape(B * S, D)
    bvh = bv.reshape(H, HD).T  # [HD, H]
    in_maps = []
    for core in range(NCORE):
        m = dict(shared)
        m["hs"] = np.ascontiguousarray(hs_flat[R * core:R * (core + 1)])
        m["bvh2"] = np.ascontiguousarray(bvh[:, 2 * core:2 * core + 2])
        in_maps.append(m)

    res = run_bass_kernel_spmd(nc, in_maps, core_ids=list(range(NCORE)))
    global LAST_RESULT
    LAST_RESULT = res

    out_full = np.empty((B * S, D), dtype=np.float32)
    for core in range(NCORE):
        out_full[R * core:R * (core + 1)] = res.results[core]["out"]
    return out_full.reshape(B, S, D)
